# revision 1
# baseline (speedup 1.0000x reference)
"""Trainium2 Bass kernel for nn_CHTransform (cylindrical-harmonics decomposition).

Math: ch[b,c,n,k,l] = dtheta*dz * sum_{r,t,z} vol[b,c,r,t,z]
                       * Wr[|n|,k,r] * e^{i n theta_t}/sqrt(2pi) * e^{i pi l z_z}/sqrt(2)

The angular basis is even (cos) / odd (sin) in n and the radial basis depends
only on |n|, so only m=|n| in 0..3 is needed: a combined host-precomputed basis
C1[rt, j] (16 cos-cols (m,k) + 12 sin-cols (m>=1,k), 28 total) contracts r and
t in one TensorE pass; the tiny z-contraction against the axial basis and the
+/-n complex unfold happen on host during the unshard (64 x 28 x 96 floats).

Device (per core: 8 of the 64 (b,c) pairs, data-parallel, no communication):
  - vol arrives as [8, 128, 6912] fp16: partition p holds 72 consecutive
    rt-rows; K-tile j of the contraction lives at free columns j*96..(j+1)*96,
    i.e. rt = p*72 + j, with C1 host-permuted to match.  fp16 halves the HBM
    stream vs f32 (the kernel is DMA-bound); rounding adds ~3e-4 rel err vs
    the 2e-2 gate, PSUM still accumulates in f32.
  - (b,c) are processed in 2 groups of 4: one matmul per K-tile j with
    lhsT = C1_j [128, 28] (28-col LDWEIGHTS) and a 3D moving operand
    [128 x 4bc x 96z] (N=384, amortizes LDWEIGHTS 4x vs per-bc) accumulating
    into one PSUM bank [28, 384] over all 72 j.
  - volumes stream in 18-K-tile grouped chunks: 512 descriptors x 3456 B,
    the measured SDMA sweet spot (~24.6 GB/s/engine; 13.8 KB runs drop to
    21.5, 1152 B to 22).  The final group tapers [18,18,15] then per-bc
    [12,6,3] chunks with per-bc PSUM-slice copy + out-DMA: chunk completions
    bunch at stream end (the slowest SDMA engine finishes every chunk late),
    so fine per-bc quanta keep the PE within ~1 us of the stream and only
    bc3's ~2.5 us copy/out chain trails the last input byte.  c1
    (128 x 4032 B) is triggered first as ring warmup.  Outputs ride the
    scalar ring: sync-ring triggers execute in program order, so an out
    trigger blocking on its copy there would stall later input triggers.
  - measured exec_time (= last instruction end - first const MEMSET) splits
    ~2.6 us counted head, 38.5-43.5 us input stream (run-to-run SDMA mode),
    ~2.5 us tail, plus a fixed ~8.5 us end-of-NEFF semaphore-teardown storm
    (255 clears of S[5..256]) that exec_time includes regardless of kernel
    structure.
"""

import math

import numpy as np

import concourse.bacc as bacc
import concourse.mybir as mybir
import concourse.tile as tile
from concourse.bass_utils import run_bass_kernel_spmd

# Problem constants (hardcoded per spec nn_CHTransform_43439299231904)
B, C, R, T, Z = 8, 8, 96, 96, 96
MAX_N, MAX_K, MAX_L = 3, 4, 5
R_SCALE = 1.0
N_CORES = 8
BC = B * C                   # 64 (b,c) pairs
BC_PER_CORE = BC // N_CORES  # 8
RT = R * T                   # 9216
P = 128                      # SBUF partitions
Q = RT // P                  # 72 rt-rows per partition = # of K-tiles
NJ = 28                      # stage-1 output columns: 16 cos (m,k) + 12 sin
NL = 22                      # host stage-2 columns: 11 cos l + 11 sin l
GRP = 4                      # (b,c) pairs per matmul group (N = GRP*Z = 384)
NGRP = BC_PER_CORE // GRP    # 2
CHUNKS = [18, 18, 18, 18]    # K-tiles per grouped DMA chunk (3456-B runs)
# last group tapers into per-bc chunks + per-bc output slices so the chain
# after the final input byte is just 3 N=96 matmuls -> slice copy -> 11 KB
# out-DMA.  (A grouped end chunk leaves ~3 us of PE+copy+out serialized at
# the end: chunk completions bunch up because the slowest SDMA engine
# finishes every chunk late.)
CHUNKS_END = [18, 18, 15]    # 51 grouped tiles on the final group
TAIL_CHUNKS = [12, 6, 3]     # then per-bc j-chunks (runs 2304/1152/576 B):
# fine-grained completion quanta keep the PE within ~1 us of the stream at
# the end instead of draining a ~3 us backlog after the last input byte
TAIL_J = sum(TAIL_CHUNKS)    # 21 per-bc tail K-tiles

BESSEL_ZEROS = {0: [2.4048, 5.5201, 8.6537, 11.7915, 14.9309],
                1: [3.8317, 7.0156, 10.1735, 13.3237, 16.4706],
                2: [5.1356, 8.4172, 11.6198, 14.796, 18.0155],
                3: [6.3802, 9.761, 13.0152, 16.2235, 19.4094]}

MM_DT = mybir.dt.float16
TRACE = False               # test harness sets True for NTFF profiling
LAST_RESULTS = None         # BassKernelResults of the most recent run


def _bessel_j(n, x):
    xs = np.maximum(x, 1e-12)
    if n == 0:
        small = np.abs(x) < 1.0
        med = (np.abs(x) >= 1.0) & (np.abs(x) < 5.0)
        sm = 1.0 - x ** 2 / 4.0 + x ** 4 / 64.0
        md = np.cos(x - np.pi / 4) / np.sqrt(xs)
        lg = np.sqrt(2.0 / (np.pi * xs)) * np.cos(x - np.pi / 4)
        return np.where(small, sm, np.where(med, md, lg))
    elif n == 1:
        small = np.abs(x) < 1.0
        med = (np.abs(x) >= 1.0) & (np.abs(x) < 5.0)
        sm = x / 2.0 - x ** 3 / 16.0
        md = np.sin(x - np.pi / 4) / np.sqrt(xs)
        lg = np.sqrt(2.0 / (np.pi * xs)) * np.cos(x - 3 * np.pi / 4)
        return np.where(small, sm, np.where(med, md, lg))
    else:
        logfact = sum(math.log(i) for i in range(1, n + 1))
        small = np.abs(x) < 0.1 * n
        sm = np.exp(n * np.log(xs / 2.0) - logfact)
        lg = np.sqrt(2.0 / (np.pi * xs)) * np.cos(x - (2 * n + 1) * np.pi / 4)
        return np.where(small, sm, lg)


def _make_basis():
    """C1_perm [128, Q*NJ] and ax_cat [Z, NL] f32; dtheta*dz folded into ax_cat."""
    r = np.linspace(0.0, 1.0, R) * R_SCALE
    theta = np.linspace(0.0, 2 * math.pi, T)
    z = np.linspace(-1.0, 1.0, Z)
    dr = R_SCALE / (R - 1)
    dtheta = 2 * math.pi / T
    dz = 2.0 / (Z - 1)
    Wm = np.zeros((4, MAX_K, R))
    for m in range(4):
        for k in range(1, MAX_K + 1):
            r_nk = BESSEL_ZEROS[m][k - 1]
            J = _bessel_j(m, r_nk * r)
            ss = (T * Z) * np.sum((J * r * dr) ** 2)
            norm = 1.0 / np.sqrt(ss) if ss > 1e-6 else 0.0
            Wm[m, k - 1] = J * norm * r * dr
    ang_scale = 1.0 / math.sqrt(2 * math.pi)
    C1 = np.zeros((RT, NJ))
    for m in range(4):
        cosm = np.cos(m * theta) * ang_scale
        sinm = np.sin(m * theta) * ang_scale
        for k in range(MAX_K):
            C1[:, m * 4 + k] = (Wm[m, k][:, None] * cosm[None, :]).reshape(-1)
            if m >= 1:
                C1[:, 16 + (m - 1) * 4 + k] = (
                    Wm[m, k][:, None] * sinm[None, :]).reshape(-1)
    # permute rows to the [128, 6912] data layout: K-tile j holds rt = p*Q + j
    C1_perm = C1.reshape(P, Q, NJ).reshape(P, Q * NJ)
    l_vals = np.arange(-MAX_L, MAX_L + 1)
    ax_scale = (1.0 / math.sqrt(2)) * dtheta * dz
    ax_cat = np.zeros((Z, NL))
    for li, lv in enumerate(l_vals):
        ax_cat[:, li] = np.cos(math.pi * lv * z) * ax_scale
        ax_cat[:, 11 + li] = np.sin(math.pi * lv * z) * ax_scale
    return (np.ascontiguousarray(C1_perm, dtype=np.float32),
            np.ascontiguousarray(ax_cat, dtype=np.float32))


def _combine(out2):
    """out2 [..., 28, 22] f32 -> ch [..., 7, 4, 11] complex64 (the +/-n unfold)."""
    lead = out2.shape[:-2]
    E = out2[..., :16, :].reshape(*lead, 4, MAX_K, 2, 11)  # cos block, q=0 re / 1 im
    O = out2[..., 16:, :].reshape(*lead, 3, MAX_K, 2, 11)  # sin block, m=1..3
    ch = np.zeros((*lead, 2 * MAX_N + 1, MAX_K, 2 * MAX_L + 1), dtype=np.complex64)
    ch[..., 3, :, :] = E[..., 0, :, 0, :] + 1j * E[..., 0, :, 1, :]
    for m in range(1, 4):
        Er, Ei = E[..., m, :, 0, :], E[..., m, :, 1, :]
        Or_, Oi = O[..., m - 1, :, 0, :], O[..., m - 1, :, 1, :]
        ch[..., 3 + m, :, :] = (Er - Oi) + 1j * (Ei + Or_)
        ch[..., 3 - m, :, :] = (Er + Oi) + 1j * (Ei - Or_)
    return ch


def _build_nc():
    f32 = mybir.dt.float32
    nc = bacc.Bacc("TRN2", target_bir_lowering=False, debug=False,
                   num_devices=N_CORES)
    vol_in = nc.dram_tensor("vol", [BC_PER_CORE, P, Q * Z], MM_DT,
                            kind="ExternalInput")
    c1_in = nc.dram_tensor("c1", [P, Q * NJ], MM_DT, kind="ExternalInput")
    out = nc.dram_tensor("out", [NGRP, NJ, GRP * Z], f32, kind="ExternalOutput")

    with tile.TileContext(nc) as tc:
        with (
            tc.tile_pool(name="consts", bufs=1) as consts,
            # enough chunk buffers that every end-taper DMA can be in flight
            # before its predecessors' matmuls retire (triggers stall on
            # buffer-recycle sems otherwise and the stream drains at the end)
            tc.tile_pool(name="vpool", bufs=5) as vpool,
            tc.tile_pool(name="vtail", bufs=3 * GRP) as vtail,
            tc.tile_pool(name="obuf", bufs=2) as obuf,
            tc.tile_pool(name="obufb", bufs=GRP) as obufb,
            tc.tile_pool(name="pspool", bufs=2, space="PSUM") as pspool,
        ):
            # c1 first: best-rate descriptors warm the ring while the PE
            # needs it before any matmul anyway.  The first trigger carries
            # only 16 partitions (one descriptor per SDMA engine, ~100 ns to
            # write) so the queue doorbell rings ~0.6 us earlier than a full
            # 128-descriptor trigger would; the rest follows immediately.
            c1_sb = consts.tile([P, Q * NJ], MM_DT)
            nc.sync.dma_start(c1_sb[:16, :], c1_in[:16, :])
            nc.sync.dma_start(c1_sb[16:, :], c1_in[16:, :])
            for g in range(NGRP):
                chunks = CHUNKS if g < NGRP - 1 else CHUNKS_END
                ps = pspool.tile([NJ, GRP * Z], f32)
                j0 = 0
                for ch in chunks:
                    v4 = vpool.tile([P, GRP * ch * Z], MM_DT,
                                    padded_shape=[P, GRP * max(CHUNKS) * Z])
                    src = (vol_in[g * GRP:(g + 1) * GRP, :,
                                  j0 * Z:(j0 + ch) * Z]
                           .rearrange("b p f -> p b f"))
                    dst = (v4[:, :GRP * ch * Z]
                           .rearrange("p (b f) -> p b f", b=GRP))
                    nc.sync.dma_start(dst, src)
                    v4r = v4[:, :GRP * ch * Z].rearrange(
                        "p (b j z) -> p b j z", b=GRP, j=ch)
                    for jj in range(ch):
                        j = j0 + jj
                        nc.tensor.matmul(
                            ps[:],
                            c1_sb[:, j * NJ:(j + 1) * NJ],
                            v4r[:, :, jj, :],
                            start=(j == 0),
                            stop=(j == Q - 1 - (TAIL_J if g == NGRP - 1 else 0)),
                        )
                    j0 += ch
                if g < NGRP - 1:
                    ob = obuf.tile([NJ, GRP * Z], f32)
                    nc.vector.tensor_copy(ob[:], ps[:])
                    # mid-stream output rides the scalar ring so its trigger
                    # never blocks pending input triggers on the sync ring
                    nc.scalar.dma_start(out[g], ob[:])
                else:
                    # per-bc tail: each bc's N=96 matmuls trail its own small
                    # DMAs, and its PSUM column slice is final once its tail
                    # retires (disjoint slices), so copy+out per bc overlap
                    # the remaining stream; only bc3's ~1.5 us chain follows
                    # the last input byte
                    # ALL tail matmuls before ANY copy: a copy reading slice b
                    # makes every matmul emitted after it (slice b+1) inherit
                    # a false tile-granular WAR dependency, stalling the PE
                    # ~0.4 us per bc right at the stream end
                    for b in range(GRP):
                        jb = j0
                        for tch in TAIL_CHUNKS:
                            vt = vtail.tile([P, tch * Z], MM_DT,
                                            padded_shape=[P, TAIL_CHUNKS[0] * Z],
                                            tag="vt")
                            nc.sync.dma_start(
                                vt[:],
                                vol_in[g * GRP + b, :, jb * Z:(jb + tch) * Z])
                            vtr = vt[:].rearrange("p (j z) -> p j z", j=tch)
                            for jj in range(tch):
                                j = jb + jj
                                nc.tensor.matmul(
                                    ps[:, b * Z:(b + 1) * Z],
                                    c1_sb[:, j * NJ:(j + 1) * NJ],
                                    vtr[:, jj, :],
                                    start=False, stop=False,
                                    skip_group_check=True,
                                )
                            jb += tch
                    # ONE copy + ONE out DMA: dependency tracking is
                    # tile-granular, so per-bc copies emitted after the
                    # matmuls all wait on the FINAL matmul anyway and
                    # serialize into 4 wake+trigger chains (~5.6 us measured);
                    # a single full-width chain costs ~2.6 us.  Copy on the
                    # scalar engine so the out trigger right after it is
                    # same-engine program order (no cross-engine hop); outs
                    # stay off the sync ring, whose in-order triggers would
                    # stall later inputs.
                    obb = obufb.tile([NJ, GRP * Z], f32, tag="ob")
                    nc.scalar.copy(obb[:], ps[:])
                    nc.scalar.dma_start(out[g], obb[:])

    nc.compile()
    return nc


_NC_CACHE = None


def _get_nc():
    global _NC_CACHE
    if _NC_CACHE is None:
        _NC_CACHE = _build_nc()
    return _NC_CACHE


def kernel(cylindrical_volume):
    global LAST_RESULTS
    vol = np.asarray(cylindrical_volume, dtype=np.float32)
    assert vol.shape == (B, C, R, T, Z), vol.shape
    c1_perm, ax_cat = _make_basis()
    vol_dev = np.ascontiguousarray(vol).reshape(BC, P, Q * Z).astype(np.float16)
    c1_perm = c1_perm.astype(np.float16)

    nc = _get_nc()
    in_maps = [
        {"vol": vol_dev[i * BC_PER_CORE:(i + 1) * BC_PER_CORE], "c1": c1_perm}
        for i in range(N_CORES)
    ]
    import os
    try:
        res = run_bass_kernel_spmd(nc, in_maps, list(range(N_CORES)),
                                   trace=TRACE)
    except ModuleNotFoundError:
        # BASS_TRACE set but this image lacks the axon NTFF hook module;
        # rerun without tracing rather than failing
        os.environ["BASS_NEVER_TRACE"] = "1"
        try:
            res = run_bass_kernel_spmd(nc, in_maps, list(range(N_CORES)),
                                       trace=False)
        finally:
            os.environ.pop("BASS_NEVER_TRACE", None)
    LAST_RESULTS = res
    # per-core out [NGRP, 28, GRP*Z] -> [8bc, 28, 96z]
    S = np.concatenate(
        [res.results[i]["out"].reshape(NGRP, NJ, GRP, Z).transpose(0, 2, 1, 3)
         .reshape(BC_PER_CORE, NJ, Z)
         for i in range(N_CORES)], axis=0)          # [64, 28, 96]
    out2 = np.einsum('bjz,zl->bjl', S, ax_cat)       # host stage 2: [64, 28, 22]
    ch = _combine(out2)
    return ch.reshape(B, C, 2 * MAX_N + 1, MAX_K, 2 * MAX_L + 1)



# revision 5
# speedup vs baseline: 1.5225x; 1.5225x over previous
"""Trainium2 Bass kernel for nn_CHTransform (cylindrical-harmonics decomposition).

Math: ch[b,c,n,k,l] = dtheta*dz * sum_{r,t,z} vol[b,c,r,t,z]
                       * Wr[|n|,k,r] * e^{i n theta_t}/sqrt(2pi) * e^{i pi l z_z}/sqrt(2)

The angular basis is even (cos) / odd (sin) in n and the radial basis depends
only on |n|, so only m=|n| in 0..3 is needed: a combined host-precomputed basis
C1[rt, j] (16 cos-cols (m,k) + 12 sin-cols (m>=1,k), 28 total) contracts r and
t in one TensorE pass; the tiny z-contraction against the axial basis and the
+/-n complex unfold happen on host during the unshard (64 x 28 x 96 floats).

fp8 scheme (the kernel is DMA-bound; e4m3 halves the HBM stream vs fp16 and
the PE runs DoubleRow fp8 matmuls at 0.5 cycles/row):
  - vol is quantized to e4m3 with FIRST-ORDER NOISE SHAPING along z (host
    error-feedback): the axial basis only probes low frequencies in z
    (|l| <= 5 -> NTF |1-z^-1| <= 0.33), so the e4m3 quantization noise
    (~2.6% rms white) is pushed out of band.  Measured end-to-end rel err
    8.9e-3 vs the 2e-2 gate (plain e4m3 rounding: 4.0e-2, fails).
  - weights: e4m3 pair W0 = q(C1*WS), W1 = q((C1*WS - W0)/2^-4) side by side
    in the stationary matrix (56 cols; PE output columns are parallel so the
    residual costs nothing); host recombines S0 + 2^-4*S1 and divides by WS.
    Weight quant error drops 3.8e-2 -> 1.2e-3 relative.
  - DoubleRow: each matmul call consumes a K-tile PAIR: lhsT [128, 2, 56],
    rhs [128, 2, GRP*Z] (the 2 sub-tiles adjacent j at stride Z in SBUF),
    out [56, GRP*Z] accumulating f32 in PSUM over 36 pairs.

Device (per core: 8 of the 64 (b,c) pairs, data-parallel, no communication):
  - vol arrives as [8, 128, 6912] e4m3: partition p holds 72 consecutive
    rt-rows; K-tile j of the contraction lives at free columns j*96..(j+1)*96,
    i.e. rt = p*72 + j, with C1 host-permuted to match.
  - (b,c) are processed in 2 groups of 4 (N = GRP*Z = 384 amortizes the
    112-col LDWEIGHTS 4x vs per-bc).
  - volumes stream in 36-K-tile grouped chunks: 512 descriptors x 3456 B,
    the measured SDMA sweet spot (~24.6 GB/s/engine).  The final group
    tapers [18g, 9g] pairs then per-bc [6, 3] pair chunks with a single
    full-width PSUM copy + out-DMA: chunk completions bunch at stream end,
    so fine per-bc quanta keep the PE within ~1 us of the stream.  c1
    (128 x 4032 B) is triggered first as ring warmup.  Outputs ride the
    scalar ring: sync-ring triggers execute in program order, so an out
    trigger blocking on its copy there would stall later input triggers.
  - measured exec_time (= last instruction end - first const MEMSET)
    includes a fixed ~8.5 us end-of-NEFF semaphore-teardown storm (255
    clears of S[5..256]) regardless of kernel structure.
"""

import math

import numpy as np
import ml_dtypes

import concourse.bacc as bacc
import concourse.mybir as mybir
import concourse.tile as tile
from concourse.bass_utils import run_bass_kernel_spmd

# Problem constants (hardcoded per spec nn_CHTransform_43439299231904)
B, C, R, T, Z = 8, 8, 96, 96, 96
MAX_N, MAX_K, MAX_L = 3, 4, 5
R_SCALE = 1.0
N_CORES = 8
BC = B * C                   # 64 (b,c) pairs
BC_PER_CORE = BC // N_CORES  # 8
RT = R * T                   # 9216
P = 128                      # SBUF partitions
Q = RT // P                  # 72 rt-rows per partition = # of K-tiles
NJ = 28                      # logical output columns: 16 cos (m,k) + 12 sin
NJ2 = 2 * NJ                 # stationary cols: [W0 | W1 residual]
NJP = 64                     # padded sub-row width: DoubleRow LDWEIGHTS needs
                             # the pair-dim AP step to be a multiple of 16 B
                             # (s3_lw_dual_fp8_restrictions), so 56 -> 64
NL = 22                      # host stage-2 columns: 11 cos l + 11 sin l
GRP = 4                      # (b,c) pairs per matmul group (N = GRP*Z = 384)
NGRP = BC_PER_CORE // GRP    # 2
PAIRS = Q // 2               # 36 DoubleRow K-tile pairs
# grouped DMA chunks in PAIRS (x2 j-tiles x 96 x 1B = 3456-B runs at 18):
CHUNK_PAIRS = [18, 18]       # full groups: 2 chunks of 36 j-tiles
CHUNK_PAIRS_END = [18, 9]    # final group grouped part (54 j-tiles)
TAIL_PAIRS = [6, 3]          # then per-bc pair-chunks (1152/576-B runs)
TAIL_NP = sum(TAIL_PAIRS)    # 9 per-bc tail pairs (18 j-tiles)
RES_S = 2.0 ** -4            # residual weight scale

BESSEL_ZEROS = {0: [2.4048, 5.5201, 8.6537, 11.7915, 14.9309],
                1: [3.8317, 7.0156, 10.1735, 13.3237, 16.4706],
                2: [5.1356, 8.4172, 11.6198, 14.796, 18.0155],
                3: [6.3802, 9.761, 13.0152, 16.2235, 19.4094]}

E4 = ml_dtypes.float8_e4m3   # == mybir.dt.np(mybir.dt.float8e4)
MM_DT = mybir.dt.float8e4
TRACE = False               # test harness sets True for NTFF profiling
LAST_RESULTS = None         # BassKernelResults of the most recent run


def _bessel_j(n, x):
    xs = np.maximum(x, 1e-12)
    if n == 0:
        small = np.abs(x) < 1.0
        med = (np.abs(x) >= 1.0) & (np.abs(x) < 5.0)
        sm = 1.0 - x ** 2 / 4.0 + x ** 4 / 64.0
        md = np.cos(x - np.pi / 4) / np.sqrt(xs)
        lg = np.sqrt(2.0 / (np.pi * xs)) * np.cos(x - np.pi / 4)
        return np.where(small, sm, np.where(med, md, lg))
    elif n == 1:
        small = np.abs(x) < 1.0
        med = (np.abs(x) >= 1.0) & (np.abs(x) < 5.0)
        sm = x / 2.0 - x ** 3 / 16.0
        md = np.sin(x - np.pi / 4) / np.sqrt(xs)
        lg = np.sqrt(2.0 / (np.pi * xs)) * np.cos(x - 3 * np.pi / 4)
        return np.where(small, sm, np.where(med, md, lg))
    else:
        logfact = sum(math.log(i) for i in range(1, n + 1))
        small = np.abs(x) < 0.1 * n
        sm = np.exp(n * np.log(xs / 2.0) - logfact)
        lg = np.sqrt(2.0 / (np.pi * xs)) * np.cos(x - (2 * n + 1) * np.pi / 4)
        return np.where(small, sm, lg)


def _make_basis():
    """C1 [RT, NJ] f32 and ax_cat [Z, NL] f32; dtheta*dz folded into ax_cat."""
    r = np.linspace(0.0, 1.0, R) * R_SCALE
    theta = np.linspace(0.0, 2 * math.pi, T)
    z = np.linspace(-1.0, 1.0, Z)
    dr = R_SCALE / (R - 1)
    dtheta = 2 * math.pi / T
    dz = 2.0 / (Z - 1)
    Wm = np.zeros((4, MAX_K, R))
    for m in range(4):
        for k in range(1, MAX_K + 1):
            r_nk = BESSEL_ZEROS[m][k - 1]
            J = _bessel_j(m, r_nk * r)
            ss = (T * Z) * np.sum((J * r * dr) ** 2)
            norm = 1.0 / np.sqrt(ss) if ss > 1e-6 else 0.0
            Wm[m, k - 1] = J * norm * r * dr
    ang_scale = 1.0 / math.sqrt(2 * math.pi)
    C1 = np.zeros((RT, NJ))
    for m in range(4):
        cosm = np.cos(m * theta) * ang_scale
        sinm = np.sin(m * theta) * ang_scale
        for k in range(MAX_K):
            C1[:, m * 4 + k] = (Wm[m, k][:, None] * cosm[None, :]).reshape(-1)
            if m >= 1:
                C1[:, 16 + (m - 1) * 4 + k] = (
                    Wm[m, k][:, None] * sinm[None, :]).reshape(-1)
    l_vals = np.arange(-MAX_L, MAX_L + 1)
    ax_scale = (1.0 / math.sqrt(2)) * dtheta * dz
    ax_cat = np.zeros((Z, NL))
    for li, lv in enumerate(l_vals):
        ax_cat[:, li] = np.cos(math.pi * lv * z) * ax_scale
        ax_cat[:, 11 + li] = np.sin(math.pi * lv * z) * ax_scale
    return C1.astype(np.float32), ax_cat.astype(np.float32)


def _pack_weights(C1):
    """e4m3 [P, PAIRS*2*NJP] DoubleRow-packed [W0|W1|pad] pairs, and wscale."""
    wmax = float(np.abs(C1).max())
    wscale = 2.0 ** math.floor(math.log2(128.0 / wmax))
    C1s = (C1 * wscale).astype(np.float32).reshape(P, Q, NJ)  # rt = p*Q + j
    W0 = C1s.astype(E4).astype(np.float32)
    W1 = ((C1s - W0) / RES_S).astype(E4).astype(np.float32)
    pack = np.zeros((P, PAIRS, 2, NJP), np.float32)
    pack[:, :, :, :NJ] = W0.reshape(P, PAIRS, 2, NJ)
    pack[:, :, :, NJ:NJ2] = W1.reshape(P, PAIRS, 2, NJ)
    return (np.ascontiguousarray(pack.reshape(P, PAIRS * 2 * NJP)).astype(E4),
            wscale)


def _encode_vol(vol):
    """[BC, RT, Z] f32 -> e4m3 with first-order error feedback along z."""
    out = np.empty(vol.shape, E4)
    carry = np.zeros(vol.shape[:2], np.float32)
    for z in range(vol.shape[2]):
        v = vol[:, :, z] + carry
        q = v.astype(E4)
        out[:, :, z] = q
        carry = v - q.astype(np.float32)
    return out


def _combine(out2):
    """out2 [..., 28, 22] f32 -> ch [..., 7, 4, 11] complex64 (the +/-n unfold)."""
    lead = out2.shape[:-2]
    E = out2[..., :16, :].reshape(*lead, 4, MAX_K, 2, 11)  # cos block, q=0 re / 1 im
    O = out2[..., 16:, :].reshape(*lead, 3, MAX_K, 2, 11)  # sin block, m=1..3
    ch = np.zeros((*lead, 2 * MAX_N + 1, MAX_K, 2 * MAX_L + 1), dtype=np.complex64)
    ch[..., 3, :, :] = E[..., 0, :, 0, :] + 1j * E[..., 0, :, 1, :]
    for m in range(1, 4):
        Er, Ei = E[..., m, :, 0, :], E[..., m, :, 1, :]
        Or_, Oi = O[..., m - 1, :, 0, :], O[..., m - 1, :, 1, :]
        ch[..., 3 + m, :, :] = (Er - Oi) + 1j * (Ei + Or_)
        ch[..., 3 - m, :, :] = (Er + Oi) + 1j * (Ei - Or_)
    return ch


def _build_nc():
    f32 = mybir.dt.float32
    DR = mybir.MatmulPerfMode.DoubleRow
    nc = bacc.Bacc("TRN2", target_bir_lowering=False, debug=False,
                   num_devices=N_CORES)
    vol_in = nc.dram_tensor("vol", [BC_PER_CORE, P, Q * Z], MM_DT,
                            kind="ExternalInput")
    c1_in = nc.dram_tensor("c1", [P, PAIRS * 2 * NJP], MM_DT,
                           kind="ExternalInput")
    out = nc.dram_tensor("out", [NGRP, NJ2, GRP * Z], f32,
                         kind="ExternalOutput")

    with tile.TileContext(nc) as tc:
        with (
            tc.tile_pool(name="consts", bufs=1) as consts,
            # all grouped chunks can be in flight at once (no recycle stalls)
            tc.tile_pool(name="vpool", bufs=4) as vpool,
            tc.tile_pool(name="vtail", bufs=len(TAIL_PAIRS) * GRP) as vtail,
            tc.tile_pool(name="obuf", bufs=2) as obuf,
            tc.tile_pool(name="obufb", bufs=2) as obufb,
            tc.tile_pool(name="pspool", bufs=2, space="PSUM") as pspool,
        ):
            # c1 first: best-rate descriptors warm the ring while the PE
            # needs it before any matmul anyway.  The first trigger carries
            # only 16 partitions (one descriptor per SDMA engine) so the
            # queue doorbell rings earlier than a full 128-descriptor
            # trigger would; the rest follows immediately.
            c1_sb = consts.tile([P, PAIRS * 2 * NJP], MM_DT)
            nc.sync.dma_start(c1_sb[:16, :], c1_in[:16, :])
            nc.sync.dma_start(c1_sb[16:, :], c1_in[16:, :])
            c1_p = c1_sb[:].rearrange("p (m two f) -> p m two f",
                                      two=2, f=NJP)
            for g in range(NGRP):
                chunks = CHUNK_PAIRS if g < NGRP - 1 else CHUNK_PAIRS_END
                grouped_pairs = sum(chunks)
                ps = pspool.tile([NJP, GRP * Z], f32)
                m0 = 0
                for chp in chunks:
                    v4 = vpool.tile([P, GRP * chp * 2 * Z], MM_DT,
                                    padded_shape=[P, GRP * max(CHUNK_PAIRS) * 2 * Z])
                    src = (vol_in[g * GRP:(g + 1) * GRP, :,
                                  m0 * 2 * Z:(m0 + chp) * 2 * Z]
                           .rearrange("b p f -> p b f"))
                    dst = (v4[:, :GRP * chp * 2 * Z]
                           .rearrange("p (b f) -> p b f", b=GRP))
                    nc.sync.dma_start(dst, src)
                    v4r = v4[:, :GRP * chp * 2 * Z].rearrange(
                        "p (b m two z) -> p m two b z", b=GRP, m=chp, two=2)
                    for mm in range(chp):
                        m = m0 + mm
                        nc.tensor.matmul(
                            ps[:],
                            c1_p[:, m, :, :],
                            v4r[:, mm, :, :, :],
                            start=(m == 0),
                            stop=(m == (PAIRS - 1 if g < NGRP - 1
                                        else grouped_pairs - 1)),
                            perf_mode=DR,
                        )
                    m0 += chp
                if g < NGRP - 1:
                    ob = obuf.tile([NJ2, GRP * Z], f32)
                    nc.vector.tensor_copy(ob[:], ps[:NJ2, :])
                    # mid-stream output rides the scalar ring so its trigger
                    # never blocks pending input triggers on the sync ring
                    nc.scalar.dma_start(out[g], ob[:])
                else:
                    # per-bc tail: each bc's matmuls trail its own small
                    # DMAs, keeping the PE within ~1 us of the stream end.
                    # ALL tail matmuls before ANY copy (a copy reading slice
                    # b would give later matmuls a false tile-granular WAR
                    # dependency and stall the PE right at the stream end).
                    for b in range(GRP):
                        mb = m0
                        for tch in TAIL_PAIRS:
                            vt = vtail.tile([P, tch * 2 * Z], MM_DT,
                                            padded_shape=[P, TAIL_PAIRS[0] * 2 * Z],
                                            tag="vt")
                            nc.sync.dma_start(
                                vt[:],
                                vol_in[g * GRP + b, :,
                                       mb * 2 * Z:(mb + tch) * 2 * Z])
                            vtr = vt[:].rearrange("p (m two z) -> p m two z",
                                                  m=tch, two=2)
                            for mm in range(tch):
                                nc.tensor.matmul(
                                    ps[:, b * Z:(b + 1) * Z],
                                    c1_p[:, mb + mm, :, :],
                                    vtr[:, mm, :, :],
                                    start=False, stop=False,
                                    perf_mode=DR,
                                    skip_group_check=True,
                                )
                            mb += tch
                    # ONE copy + ONE out DMA on the scalar engine: per-bc
                    # copies would all wait on the final matmul anyway
                    # (tile-granular deps) and serialize into 4 wake+trigger
                    # chains; a single full-width chain is ~2x cheaper.
                    obb = obufb.tile([NJ2, GRP * Z], f32, tag="ob")
                    nc.scalar.copy(obb[:], ps[:NJ2, :])
                    nc.scalar.dma_start(out[g], obb[:])

    nc.compile()
    return nc


_NC_CACHE = None


def _get_nc():
    global _NC_CACHE
    if _NC_CACHE is None:
        _NC_CACHE = _build_nc()
    return _NC_CACHE


def kernel(cylindrical_volume):
    global LAST_RESULTS
    vol = np.asarray(cylindrical_volume, dtype=np.float32)
    assert vol.shape == (B, C, R, T, Z), vol.shape
    C1, ax_cat = _make_basis()
    c1_pack, wscale = _pack_weights(C1)
    vol_dev = _encode_vol(
        np.ascontiguousarray(vol).reshape(BC, RT, Z)).reshape(BC, P, Q * Z)

    nc = _get_nc()
    in_maps = [
        {"vol": vol_dev[i * BC_PER_CORE:(i + 1) * BC_PER_CORE],
         "c1": c1_pack}
        for i in range(N_CORES)
    ]
    import os
    try:
        res = run_bass_kernel_spmd(nc, in_maps, list(range(N_CORES)),
                                   trace=TRACE)
    except ModuleNotFoundError:
        # BASS_TRACE set but this image lacks the axon NTFF hook module;
        # rerun without tracing rather than failing
        os.environ["BASS_NEVER_TRACE"] = "1"
        try:
            res = run_bass_kernel_spmd(nc, in_maps, list(range(N_CORES)),
                                       trace=False)
        finally:
            os.environ.pop("BASS_NEVER_TRACE", None)
    LAST_RESULTS = res
    # per-core out [NGRP, 56, GRP*Z] -> [8bc, 56, 96z]
    S = np.concatenate(
        [res.results[i]["out"].reshape(NGRP, NJ2, GRP, Z).transpose(0, 2, 1, 3)
         .reshape(BC_PER_CORE, NJ2, Z)
         for i in range(N_CORES)], axis=0)            # [64, 56, 96]
    S_eff = S[:, :NJ, :] + RES_S * S[:, NJ:, :]       # residual recombine
    out2 = np.einsum('bjz,zl->bjl', S_eff, ax_cat / wscale)  # [64, 28, 22]
    ch = _combine(out2)
    return ch.reshape(B, C, 2 * MAX_N + 1, MAX_K, 2 * MAX_L + 1)
